# revision 1
# baseline (speedup 1.0000x reference)
"""Trainium2 Bass kernel for nn_Block_16621523436203 (Mamba-style block).

Sharding: pure data-parallel — batch B=8, one batch element per NeuronCore,
no collectives.  Weights are preprocessed (transposed / LN-folded / cast) on
host; each core runs the full block for its batch element.
"""

import sys

sys.path.insert(0, "/opt/trn_rl_repo")

import math
import os

import ml_dtypes
import numpy as np

import concourse.bacc as bacc
import concourse.bass as bass
import concourse.mybir as mybir
import concourse.tile as tile

F32 = mybir.dt.float32
F32R = mybir.dt.float32r
BF16 = mybir.dt.bfloat16
AF = mybir.ActivationFunctionType
ALU = mybir.AluOpType

B, L, D = 8, 1024, 512
E = 1024  # d_inner
D2 = 512  # per-branch channels
R = 32  # dt_rank
NS = 16  # d_state
KC = 4  # conv kernel size
H = 2048  # mlp hidden
NCORES = 8
TT = L // 128  # 8 token tiles
DC = D // 128  # 4 d_model chunks
D2T = D2 // 128  # 4 channel tiles
ET = E // 128  # 8 d_inner tiles
HT = H // 128  # 16 hidden tiles
NG = 64  # scan groups: each = 8 channels x 16 states
EPS = 1e-5

_BF = ml_dtypes.bfloat16


def _f32r(ap):
    return ap.bitcast(F32R)


STOP_AFTER = int(os.environ.get("KSTOP", "3"))
KREPEAT = int(os.environ.get("KREPEAT", "1"))
KALLOC = int(os.environ.get("KALLOC", "0")) or KREPEAT


def build_kernel():
    nc = bacc.Bacc("TRN2", target_bir_lowering=False, debug=False, num_devices=1)

    din = {}

    def inp(name, shape, dtype):
        din[name] = nc.dram_tensor(name, list(shape), dtype, kind="ExternalInput")
        return din[name]

    inp("xin", (KALLOC * L, D), F32)
    inp("w_inT", (D, E), F32R)  # ln1-folded in_proj weight, transposed
    inp("c_in", (128, ET), F32)  # in_proj bias column per e-tile (W' @ ln1_b)
    inp("diag_x", (D2T * KC, 128, 128), F32R)  # conv_x diag matrices [dt*4+j]
    inp("diag_z", (D2T * KC, 128, 128), F32R)
    inp("x_projT", (D2, R + 2 * NS), F32R)
    inp("dt_projT", (R, D2), F32R)
    inp("dt_bias", (128, D2T), F32)
    inp("A_perm", (128, NG), F32)  # A[d(p), n(p)] per group column
    inp("D_col", (128, D2T), F32)
    inp("rep", (16, 128, 128), F32R)  # delta broadcast matmul: REP[q]
    inp("sel", (16, 128, 128), BF16)  # n-reduction matmul: SEL[q]
    inp("out_projT", (E, D), BF16)
    inp("fc1T", (D, H), BF16)  # ln2-folded fc1 weight, transposed
    inp("c_fc1", (128, HT), F32)  # fc1' @ ln2_b + fc1_b per h-tile
    inp("fc2T", (H, D), BF16)
    inp("fc2b", (1, D), F32R)
    inp("ident_bf", (128, 128), BF16)
    inp("zpad", (128, 3), F32R)
    inp("ones1d", (1, 128), F32R)
    inp("rep_b", (2 * NS, 128), BF16)
    inp("rep_c", (2 * NS, 128), BF16)
    inp("ident_f", (128, 128), F32)

    out_d = nc.dram_tensor("out", [KALLOC * L, D], F32, kind="ExternalOutput")
    bc_dram = nc.dram_tensor("bc_scratch", [2 * NS, L], BF16)  # Internal

    with tile.TileContext(nc) as tc:
        for rep_i in range(KREPEAT):
            _body(nc, tc, din, out_d, bc_dram, rep_i * L)
    nc.compile()
    return nc


def _body(nc, tc, din, out_d, bc_dram, row0=0):
    xin = din["xin"].ap()[row0 : row0 + L, :]
    out_ap = out_d.ap()[row0 : row0 + L, :]

    # ---------- persistent pools (cross phase) ----------
    with (
        tc.tile_pool(name="p13", bufs=1) as p13,  # crosses into phase 3
        tc.tile_pool(name="p12", bufs=1) as p12,  # dies after phase 2
    ):
        # phase1->3 tensors
        zh = [p13.tile([128, L], BF16, name=f"zh{i}", tag=f"zh{i}") for i in range(D2T)]
        y_cm = [p13.tile([128, L], BF16, name=f"ycm{i}", tag=f"ycm{i}") for i in range(D2T)]
        h_res = [p13.tile([128, D], F32, name=f"hres{i}", tag=f"hres{i}") for i in range(TT)]
        # phase1->2 tensors
        xh = [p12.tile([128, L], F32R, name=f"xh{i}", tag=f"xh{i}") for i in range(D2T)]
        delta = [p12.tile([128, L], F32R, name=f"dl{i}", tag=f"dl{i}") for i in range(D2T)]
        du = [p12.tile([128, L], BF16, name=f"du{i}", tag=f"du{i}") for i in range(D2T)]
        bbc = p12.tile([128, L], BF16, name="bbc", tag="bbc")
        cbc = p12.tile([128, L], BF16, name="cbc", tag="cbc")
        xdbl_dt = p12.tile([R, L], F32R, name="xdbl", tag="xdbl")
        a_perm = p12.tile([128, NG], F32, name="aperm", tag="aperm")
        d_col = p12.tile([128, D2T], F32, name="dcol", tag="dcol")
        dt_bias = p12.tile([128, D2T], F32, name="dtb", tag="dtb")
        c_in = p12.tile([128, ET], F32, name="cin", tag="cin")

        nc.sync.dma_start(out=a_perm[:, :], in_=din["A_perm"].ap()[:, :])
        nc.sync.dma_start(out=d_col[:, :], in_=din["D_col"].ap()[:, :])
        nc.sync.dma_start(out=dt_bias[:, :], in_=din["dt_bias"].ap()[:, :])
        nc.sync.dma_start(out=c_in[:, :], in_=din["c_in"].ap()[:, :])
        eps_t = p12.tile([128, 1], F32, name="eps_t", tag="eps_t")
        nc.vector.memset(eps_t[:, :], EPS)

        # ================= PHASE 1: LN1, in_proj, conv, x_proj, dt_proj ==========
        with (
            tc.tile_pool(name="wE", bufs=1) as wE,
            tc.tile_pool(name="xpP", bufs=1) as xpP,
            tc.tile_pool(name="t1", bufs=3) as t1,
            tc.tile_pool(name="xhatT_p", bufs=1) as xhatT_p,
            tc.tile_pool(name="psG", bufs=4, space="PSUM") as psG,
            tc.tile_pool(name="psConv", bufs=2, space="PSUM") as psConv,
            tc.tile_pool(name="psMisc", bufs=2, space="PSUM") as psMisc,
        ):
            w_inT = [wE.tile([128, E], F32R, name=f"winT{i}", tag=f"winT{i}") for i in range(DC)]
            for dc in range(DC):
                nc.sync.dma_start(
                    out=w_inT[dc][:, :], in_=din["w_inT"].ap()[dc * 128 : (dc + 1) * 128, :]
                )
            diag = {}
            for br in ("x", "z"):
                diag[br] = [
                    wE.tile([128, 128], F32R, name=f"diag{br}{i}", tag=f"diag{br}{i}") for i in range(D2T * KC)
                ]
                for i in range(D2T * KC):
                    nc.sync.dma_start(
                        out=diag[br][i][:, :], in_=din[f"diag_{br}"].ap()[i, :, :]
                    )
            x_projT = [wE.tile([128, R + 2 * NS], F32R, name=f"xpj{i}", tag=f"xpj{i}") for i in range(D2T)]
            for dt in range(D2T):
                nc.sync.dma_start(
                    out=x_projT[dt][:, :],
                    in_=din["x_projT"].ap()[dt * 128 : (dt + 1) * 128, :],
                )
            dt_projT = wE.tile([R, D2], F32R, name="dtpj", tag="dtpj")
            nc.sync.dma_start(out=dt_projT[:, :], in_=din["dt_projT"].ap()[:, :])
            ident_f = wE.tile([128, 128], F32, name="idf", tag="idf")
            nc.sync.dma_start(out=ident_f[:, :], in_=din["ident_f"].ap()[:, :])

            xhatT = [xhatT_p.tile([128, L], F32R, name=f"xhT{i}", tag=f"xhT{i}") for i in range(DC)]

            # ---- LN1 (token-major) + transpose ----
            for tt in range(TT):
                x_t = t1.tile([128, D], F32, name="x_t", tag="x_t")
                nc.sync.dma_start(out=x_t[:, :], in_=xin[tt * 128 : (tt + 1) * 128, :])
                stats = t1.tile([128, 6], F32, name="stats", tag="stats")
                nc.vector.bn_stats(out=stats[:, :], in_=x_t[:, :])
                mv = t1.tile([128, 2], F32, name="mv", tag="mv")
                nc.vector.bn_aggr(out=mv[:, :], in_=stats[:, :])
                lv = t1.tile([128, 1], F32, name="lv", tag="lv")
                nc.scalar.activation(
                    out=lv[:, :], in_=mv[:, 1:2], func=AF.Ln, bias=eps_t[:, :], scale=1.0
                )
                r_t = t1.tile([128, 1], F32, name="r_t", tag="r_t")
                nc.scalar.activation(
                    out=r_t[:, :], in_=lv[:, :], func=AF.Exp, bias=0.0, scale=-0.5
                )
                xhat = t1.tile([128, D], F32, name="xhat", tag="xhat")
                nc.vector.tensor_scalar(
                    out=xhat[:, :],
                    in0=x_t[:, :],
                    scalar1=mv[:, 0:1],
                    scalar2=r_t[:, :],
                    op0=ALU.subtract,
                    op1=ALU.mult,
                )
                # transpose 4 blocks of [128,128] into xhatT[dc][:, tt*128:+128]
                for dc in range(DC):
                    ps_tr = psMisc.tile([128, 128], F32, name="ps_tr", tag="m")
                    nc.tensor.transpose(
                        ps_tr[:, :], xhat[:, dc * 128 : (dc + 1) * 128], ident_f[:, :]
                    )
                    nc.scalar.copy(
                        out=xhatT[dc][:, tt * 128 : (tt + 1) * 128], in_=ps_tr[:, :]
                    )

            # ---- conv input buffers (padded by 1 left / 2 right) ----
            xp = {
                "x": [xpP.tile([128, L + 3], F32R, name=f"xpx{i}", tag=f"xpx{i}") for i in range(D2T)],
                "z": [xpP.tile([128, L + 3], F32R, name=f"xpz{i}", tag=f"xpz{i}") for i in range(D2T)],
            }
            for br in ("x", "z"):
                for dt in range(D2T):
                    nc.sync.dma_start(
                        out=xp[br][dt][:, 0:1], in_=din["zpad"].ap()[:, 0:1]
                    )
                    nc.sync.dma_start(
                        out=xp[br][dt][:, L + 1 : L + 3], in_=din["zpad"].ap()[:, 0:2]
                    )

            # ---- in_proj: xzT[e, l] = W' @ xhatT  (+ c_in) ----
            for et in range(ET):
                for lc in range(2):
                    ps = psG.tile([128, 512], F32, name="ps_inp", tag="ps_inp")
                    for dc in range(DC):
                        nc.tensor.matmul(
                            ps[:, :],
                            _f32r(w_inT[dc][:, et * 128 : (et + 1) * 128]),
                            _f32r(xhatT[dc][:, lc * 512 : (lc + 1) * 512]),
                            start=(dc == 0),
                            stop=(dc == DC - 1),
                        )
                    br, dt = ("x", et) if et < D2T else ("z", et - D2T)
                    nc.scalar.activation(
                        out=xp[br][dt][:, 1 + lc * 512 : 1 + (lc + 1) * 512],
                        in_=ps[:, :],
                        func=AF.Identity,
                        bias=c_in[:, et : et + 1],
                        scale=1.0,
                    )

            # ---- depthwise conv (4 diagonal matmuls) + SiLU ----
            for br in ("x", "z"):
                for dt in range(D2T):
                    for lc in range(2):
                        ps = psConv.tile([128, 512], F32, name="ps_conv", tag="ps_conv")
                        for j in range(KC):
                            nc.tensor.matmul(
                                ps[:, :],
                                diag[br][dt * KC + j][:, :],
                                xp[br][dt][:, lc * 512 + j : lc * 512 + j + 512],
                                start=(j == 0),
                                stop=(j == KC - 1),
                            )
                        if br == "x":
                            nc.scalar.activation(
                                out=xh[dt][:, lc * 512 : (lc + 1) * 512],
                                in_=ps[:, :],
                                func=AF.Silu,
                                bias=0.0,
                                scale=1.0,
                            )
                        else:
                            nc.scalar.activation(
                                out=zh[dt][:, lc * 512 : (lc + 1) * 512],
                                in_=ps[:, :],
                                func=AF.Silu,
                                bias=0.0,
                                scale=1.0,
                            )

            # ---- x_proj: x_dbl[r, l] = x_projT.T @ xh ----
            bc_sb = t1.tile([2 * NS, L], BF16, name="bc_sb", tag="bc_sb")
            for lc in range(2):
                ps = psMisc.tile([R + 2 * NS, 512], F32, name="ps_xdbl", tag="m")
                for dt in range(D2T):
                    nc.tensor.matmul(
                        ps[:, :],
                        x_projT[dt][:, :],
                        _f32r(xh[dt][:, lc * 512 : (lc + 1) * 512]),
                        start=(dt == 0),
                        stop=(dt == D2T - 1),
                    )
                nc.scalar.copy(
                    out=xdbl_dt[:, lc * 512 : (lc + 1) * 512], in_=ps[0:R, :]
                )
                nc.vector.tensor_copy(
                    bc_sb[:, lc * 512 : (lc + 1) * 512], ps[R : R + 2 * NS, :]
                )
            # broadcast B and C across the 8-channel groups via PE selection
            rep_b = wE.tile([2 * NS, 128], BF16, name="rep_b", tag="rep_b")
            rep_c = wE.tile([2 * NS, 128], BF16, name="rep_c", tag="rep_c")
            nc.sync.dma_start(out=rep_b[:, :], in_=din["rep_b"].ap()[:, :])
            nc.sync.dma_start(out=rep_c[:, :], in_=din["rep_c"].ap()[:, :])
            for dst_t, rep_t in ((bbc, rep_b), (cbc, rep_c)):
                for lc in range(2):
                    ps = psMisc.tile([128, 512], F32, name="ps_bc", tag="m")
                    nc.tensor.matmul(
                        ps[:, :],
                        rep_t[:, :],
                        bc_sb[:, lc * 512 : (lc + 1) * 512],
                        start=True,
                        stop=True,
                    )
                    nc.vector.tensor_copy(
                        dst_t[:, lc * 512 : (lc + 1) * 512], ps[:, :]
                    )

            # ---- dt_proj + softplus -> delta ; du = delta * xh ----
            # All 8 Exps are emitted before all 8 Lns so the ACT LUT table
            # switches twice per cluster instead of twice per pair.
            t_sps = {}
            for dt in range(D2T):
                for lc in range(2):
                    ps = psMisc.tile([128, 512], F32, name="ps_dt", tag="m")
                    nc.tensor.matmul(
                        ps[:, :],
                        _f32r(dt_projT[:, dt * 128 : (dt + 1) * 128]),
                        _f32r(xdbl_dt[:, lc * 512 : (lc + 1) * 512]),
                        start=True,
                        stop=True,
                    )
                    t_sp = t1.tile(
                        [128, 512], F32, name=f"tsp{dt}{lc}", tag=f"tsp{dt}{lc}", bufs=1
                    )
                    nc.scalar.activation(
                        out=t_sp[:, :],
                        in_=ps[:, :],
                        func=AF.Exp,
                        bias=dt_bias[:, dt : dt + 1],
                        scale=1.0,
                    )
                    t_sps[(dt, lc)] = t_sp
            for dt in range(D2T):
                for lc in range(2):
                    nc.scalar.activation(
                        out=delta[dt][:, lc * 512 : (lc + 1) * 512],
                        in_=t_sps[(dt, lc)][:, :],
                        func=AF.Ln,
                        bias=1.0,
                        scale=1.0,
                    )
                nc.vector.tensor_tensor(
                    out=du[dt][:, :],
                    in0=delta[dt][:, :].bitcast(F32),
                    in1=xh[dt][:, :].bitcast(F32),
                    op=ALU.mult,
                )

        if STOP_AFTER == 1:
            for dt in range(D2T):
                nc.sync.dma_start(
                    out=out_ap[dt * 128 : (dt + 1) * 128, :],
                    in_=delta[dt][:, 0:512].bitcast(F32),
                )
                nc.sync.dma_start(
                    out=out_ap[512 + dt * 128 : 512 + (dt + 1) * 128, :],
                    in_=xh[dt][:, 0:512].bitcast(F32),
                )
            return

        # ================= PHASE 2: selective scan ==========
        with tc.tile_pool(name="wL", bufs=1) as wL:
            rep = [wL.tile([128, 128], F32R, name=f"rep{q}", tag=f"rep{q}") for q in range(16)]
            sel = [wL.tile([128, 128], BF16, name=f"sel{q}", tag=f"sel{q}") for q in range(16)]
            for q in range(16):
                nc.sync.dma_start(out=rep[q][:, :], in_=din["rep"].ap()[q, :, :])
                nc.sync.dma_start(out=sel[q][:, :], in_=din["sel"].ap()[q, :, :])
            out_projT = [wL.tile([128, D], BF16, name=f"opT{i}", tag=f"opT{i}") for i in range(ET)]
            for k in range(ET):
                nc.sync.dma_start(
                    out=out_projT[k][:, :],
                    in_=din["out_projT"].ap()[k * 128 : (k + 1) * 128, :],
                )
            fc1T = [wL.tile([128, H], BF16, name=f"fc1T{i}", tag=f"fc1T{i}") for i in range(DC)]
            for dc in range(DC):
                nc.sync.dma_start(
                    out=fc1T[dc][:, :], in_=din["fc1T"].ap()[dc * 128 : (dc + 1) * 128, :]
                )
            fc2T = [wL.tile([128, D], BF16, name=f"fc2T{i}", tag=f"fc2T{i}") for i in range(HT)]
            for ht in range(HT):
                nc.sync.dma_start(
                    out=fc2T[ht][:, :], in_=din["fc2T"].ap()[ht * 128 : (ht + 1) * 128, :]
                )
            c_fc1 = wL.tile([128, HT], F32, name="cfc1", tag="cfc1")
            nc.sync.dma_start(out=c_fc1[:, :], in_=din["c_fc1"].ap()[:, :])
            fc2b = wL.tile([1, D], F32R, name="fc2b", tag="fc2b")
            nc.sync.dma_start(out=fc2b[:, :], in_=din["fc2b"].ap()[:, :])
            ones1 = wL.tile([1, 128], F32R, name="ones1", tag="ones1")
            nc.sync.dma_start(out=ones1[:, :], in_=din["ones1d"].ap()[:, :])
            ident_bf = wL.tile([128, 128], BF16, name="idbf", tag="idbf")
            nc.sync.dma_start(out=ident_bf[:, :], in_=din["ident_bf"].ap()[:, :])

            with (
                tc.tile_pool(name="scanp", bufs=3) as scanp,
                tc.tile_pool(name="psDelta", bufs=2, space="PSUM") as psDelta,
                tc.tile_pool(name="psY", bufs=4, space="PSUM") as psY,
            ):
                for dt in range(D2T):
                    ps_y = [psY.tile([128, 512], F32, name="ps_y", tag="ps_y") for _ in range(2)]
                    for q in range(16):
                        g = dt * 16 + q
                        # delta broadcast via PE: psD[p, l] = delta[dt][q*8 + p//16, l]
                        ps_d = psDelta.tile([128, L], F32, name="ps_d", tag="ps_d")
                        for lc in range(2):
                            nc.tensor.matmul(
                                ps_d[:, lc * 512 : (lc + 1) * 512],
                                rep[q][:, :],
                                _f32r(delta[dt][:, lc * 512 : (lc + 1) * 512]),
                                start=True,
                                stop=True,
                            )
                        dA = scanp.tile([128, L], F32, name="dA", tag="dA")
                        nc.scalar.activation(
                            out=dA[:, :],
                            in_=ps_d[:, :],
                            func=AF.Exp,
                            bias=0.0,
                            scale=a_perm[:, g : g + 1],
                        )
                        # du broadcast via SBUF->SBUF DMA
                        dubc = scanp.tile([128, L], BF16, name="dubc", tag="dubc")
                        nc.sync.dma_start(
                            out=dubc[:, :],
                            in_=du[dt][q * 8 : (q + 1) * 8, :]
                            .unsqueeze(1)
                            .broadcast_to([8, NS, L]),
                        )
                        dBu = scanp.tile([128, L], BF16, name="dBu", tag="dBu")
                        nc.vector.tensor_tensor(
                            out=dBu[:, :], in0=dubc[:, :], in1=bbc[:, :], op=ALU.mult
                        )
                        hs = scanp.tile([128, L], BF16, name="hs", tag="hs")
                        nc.vector.tensor_tensor_scan(
                            hs[:, :], dA[:, :], dBu[:, :], 0.0, ALU.mult, ALU.add
                        )
                        yt = scanp.tile([128, L], BF16, name="yt", tag="yt")
                        nc.vector.tensor_tensor(
                            out=yt[:, :], in0=hs[:, :], in1=cbc[:, :], op=ALU.mult
                        )
                        for lc in range(2):
                            nc.tensor.matmul(
                                ps_y[lc][:, :],
                                sel[q][:, :],
                                yt[:, lc * 512 : (lc + 1) * 512],
                                start=(q == 0),
                                stop=(q == 15),
                            )
                    # evac: y_cm = y_ssm + D * u
                    for lc in range(2):
                        nc.vector.scalar_tensor_tensor(
                            out=y_cm[dt][:, lc * 512 : (lc + 1) * 512],
                            in0=xh[dt][:, lc * 512 : (lc + 1) * 512].bitcast(F32),
                            scalar=d_col[:, dt : dt + 1],
                            in1=ps_y[lc][:, :],
                            op0=ALU.mult,
                            op1=ALU.add,
                        )

            if STOP_AFTER == 2:
                for dt in range(D2T):
                    nc.gpsimd.dma_start(
                        out=out_ap[dt * 128 : (dt + 1) * 128, 0:256],
                        in_=y_cm[dt][:, 0:256],
                    )
                return

            # ================= PHASE 3: out_proj, LN2, MLP ==========
            with (
                tc.tile_pool(name="p3", bufs=1) as p3,
                tc.tile_pool(name="t3", bufs=3) as t3,
                tc.tile_pool(name="psG3", bufs=4, space="PSUM") as psG3,
                tc.tile_pool(name="psTr", bufs=2, space="PSUM") as psTr,
            ):
                xhat2 = [p3.tile([128, D], BF16, name=f"xh2{i}", tag=f"xh2{i}") for i in range(TT)]
                eps3 = p3.tile([128, 1], F32, name="eps3", tag="eps3")
                nc.vector.memset(eps3[:, :], EPS)
                xhat2T = [p3.tile([128, L], BF16, name=f"xh2T{i}", tag=f"xh2T{i}") for i in range(DC)]
                aT = [p3.tile([128, L], BF16, name=f"aT{i}", tag=f"aT{i}") for i in range(HT)]

                # ---- out_proj + residual 1 + LN2 prep ----
                for tt in range(TT):
                    ps = psG3.tile([128, D], F32, name="ps_op", tag="g3")
                    korder = list(range(D2T, ET)) + list(range(D2T))
                    for ki, k in enumerate(korder):
                        lhs = (
                            y_cm[k][:, tt * 128 : (tt + 1) * 128]
                            if k < D2T
                            else zh[k - D2T][:, tt * 128 : (tt + 1) * 128]
                        )
                        nc.tensor.matmul(
                            ps[:, :],
                            lhs,
                            out_projT[k][:, :],
                            start=(ki == 0),
                            stop=(ki == ET - 1),
                        )
                    x_t = t3.tile([128, D], F32, name="x_t3", tag="x_t3")
                    nc.sync.dma_start(
                        out=x_t[:, :], in_=xin[tt * 128 : (tt + 1) * 128, :]
                    )
                    nc.vector.tensor_tensor(
                        out=h_res[tt][:, :], in0=ps[:, :], in1=x_t[:, :], op=ALU.add
                    )
                    # LN2
                    stats = t3.tile([128, 6], F32, name="stats3", tag="stats3")
                    nc.vector.bn_stats(out=stats[:, :], in_=h_res[tt][:, :])
                    mv = t3.tile([128, 2], F32, name="mv3", tag="mv3")
                    nc.vector.bn_aggr(out=mv[:, :], in_=stats[:, :])
                    lv = t3.tile([128, 1], F32, name="lv3", tag="lv3")
                    nc.scalar.activation(
                        out=lv[:, :], in_=mv[:, 1:2], func=AF.Ln, bias=eps3[:, :], scale=1.0
                    )
                    r_t = t3.tile([128, 1], F32, name="r3", tag="r3")
                    nc.scalar.activation(
                        out=r_t[:, :], in_=lv[:, :], func=AF.Exp, bias=0.0, scale=-0.5
                    )
                    nc.vector.tensor_scalar(
                        out=xhat2[tt][:, :],
                        in0=h_res[tt][:, :],
                        scalar1=mv[:, 0:1],
                        scalar2=r_t[:, :],
                        op0=ALU.subtract,
                        op1=ALU.mult,
                    )

                if STOP_AFTER == 21:
                    for tt in range(TT):
                        nc.sync.dma_start(
                            out=out_ap[tt * 128 : (tt + 1) * 128, :],
                            in_=h_res[tt][:, :],
                        )
                    return

                # ---- transpose xhat2 -> xhat2T (bf16) ----
                for dc in range(DC):
                    for half in range(2):
                        ps_t = psTr.tile([128, 512], BF16, name="ps_t3", tag="ps_t3")
                        for b4 in range(4):
                            tt = half * 4 + b4
                            nc.tensor.transpose(
                                ps_t[:, b4 * 128 : (b4 + 1) * 128],
                                xhat2[tt][:, dc * 128 : (dc + 1) * 128],
                                ident_bf[:, :],
                            )
                        nc.vector.tensor_copy(
                            xhat2T[dc][:, half * 512 : (half + 1) * 512],
                            ps_t[:, :],
                        )

                if STOP_AFTER == 22:
                    for tt in range(TT):
                        nc.sync.dma_start(
                            out=out_ap[tt * 128 : (tt + 1) * 128, :],
                            in_=h_res[tt][:, :],
                        )
                    return

                # ---- fc1 + gelu (channel-major out) ----
                for ht in range(HT):
                    for lc in range(2):
                        ps = psG3.tile([128, 512], F32, name="ps_fc1", tag="g3")
                        for dc in range(DC):
                            nc.tensor.matmul(
                                ps[:, :],
                                fc1T[dc][:, ht * 128 : (ht + 1) * 128],
                                xhat2T[dc][:, lc * 512 : (lc + 1) * 512],
                                start=(dc == 0),
                                stop=(dc == DC - 1),
                            )
                        nc.scalar.activation(
                            out=aT[ht][:, lc * 512 : (lc + 1) * 512],
                            in_=ps[:, :],
                            func=AF.Gelu,
                            bias=c_fc1[:, ht : ht + 1],
                            scale=1.0,
                        )

                if STOP_AFTER == 23:
                    for tt in range(TT):
                        nc.sync.dma_start(
                            out=out_ap[tt * 128 : (tt + 1) * 128, :],
                            in_=h_res[tt][:, :],
                        )
                    return

                # ---- fc2 + bias + residual 2 -> out ----
                for tt in range(TT):
                    ps = psG3.tile([128, D], F32, name="ps_fc2", tag="g3")
                    for ht in range(HT):
                        nc.tensor.matmul(
                            ps[:, :],
                            aT[ht][:, tt * 128 : (tt + 1) * 128],
                            fc2T[ht][:, :],
                            start=(ht == 0),
                            stop=False,
                        )
                    nc.tensor.matmul(
                        ps[:, :],
                        ones1[:, :],
                        fc2b[:, :],
                        start=False,
                        stop=True,
                    )
                    o_t = t3.tile([128, D], F32, name="o_t", tag="o_t")
                    nc.vector.tensor_tensor(
                        out=o_t[:, :], in0=ps[:, :], in1=h_res[tt][:, :], op=ALU.add
                    )
                    nc.sync.dma_start(
                        out=out_ap[tt * 128 : (tt + 1) * 128, :], in_=o_t[:, :]
                    )


def _mk_repbc(row0):
    m = np.zeros((2 * NS, 128), np.float32)
    p = np.arange(128)
    m[row0 + (p % 16), p] = 1.0
    return m


def prep_inputs(inputs):
    """Host-side weight preprocessing. Returns the shared (non-x) in_map."""
    g = {k: np.asarray(v, dtype=np.float32) for k, v in inputs.items()}

    ln1_w, ln1_b = g["ln1_w"], g["ln1_b"]
    ln2_w, ln2_b = g["ln2_w"], g["ln2_b"]

    w_in = g["in_proj_w"] * ln1_w[None, :]  # [E, D]
    c_in = (g["in_proj_w"] @ ln1_b).astype(np.float32)  # [E]

    fc1 = g["fc1_w"] * ln2_w[None, :]  # [H, D]
    c_fc1 = (g["fc1_w"] @ ln2_b + g["fc1_b"]).astype(np.float32)  # [H]

    A = -np.exp(g["A_log"])  # [D2, NS]
    # A_perm[p, g] = A[g*8 + p//16, p%16]
    p = np.arange(128)
    gg = np.arange(NG)
    A_perm = A[(gg[None, :] * 8 + (p // 16)[:, None]), (p % 16)[:, None]].astype(
        np.float32
    )

    # REP[q][k, m] = 1 iff k == q*8 + m//16   (delta row broadcast)
    rep = np.zeros((16, 128, 128), np.float32)
    for q in range(16):
        m = np.arange(128)
        rep[q, q * 8 + m // 16, m] = 1.0
    # SEL[q][k, m] = 1 iff m == q*8 + k//16   (sum over n into channel rows)
    sel = np.transpose(rep, (0, 2, 1)).copy()

    conv_x = g["conv_x_w"][:, 0, :]  # [D2, KC]
    conv_z = g["conv_z_w"][:, 0, :]
    diag_x = np.zeros((D2T * KC, 128, 128), np.float32)
    diag_z = np.zeros((D2T * KC, 128, 128), np.float32)
    idx = np.arange(128)
    for dt in range(D2T):
        for j in range(KC):
            diag_x[dt * KC + j, idx, idx] = conv_x[dt * 128 : (dt + 1) * 128, j]
            diag_z[dt * KC + j, idx, idx] = conv_z[dt * 128 : (dt + 1) * 128, j]

    def bf(x):
        return np.ascontiguousarray(x.astype(_BF))

    f = np.ascontiguousarray
    shared = {
        "w_inT": f(w_in.T),
        "c_in": f(c_in.reshape(ET, 128).T),
        "diag_x": diag_x,
        "diag_z": diag_z,
        "x_projT": f(g["x_proj_w"].T),
        "dt_projT": f(g["dt_proj_w"].T),
        "dt_bias": f(g["dt_proj_b"].reshape(D2T, 128).T),
        "A_perm": f(A_perm),
        "D_col": f(g["ssm_D"].reshape(D2T, 128).T),
        "rep": rep,
        "sel": bf(sel),
        "out_projT": bf(g["out_proj_w"].T),
        "fc1T": bf(fc1.T),
        "c_fc1": f(c_fc1.reshape(HT, 128).T),
        "fc2T": bf(g["fc2_w"].T),
        "fc2b": f(g["fc2_b"].reshape(1, D)),
        "ident_bf": bf(np.eye(128, dtype=np.float32)),
        "zpad": np.zeros((128, 3), np.float32),
        "ones1d": np.ones((1, 128), np.float32),
        "rep_b": bf(_mk_repbc(0)),
        "rep_c": bf(_mk_repbc(NS)),
        "ident_f": np.eye(128, dtype=np.float32),
    }
    return shared


_CACHED_NC = None


def kernel(**inputs):
    global _CACHED_NC
    from concourse.bass_utils import run_bass_kernel_spmd

    if _CACHED_NC is None:
        _CACHED_NC = build_kernel()
    nc = _CACHED_NC

    shared = prep_inputs(inputs)
    x = np.asarray(inputs["x"], dtype=np.float32)
    in_maps = [
        dict(
            shared,
            xin=np.ascontiguousarray(
                np.concatenate([x[i]] * KREPEAT, axis=0)
            ),
        )
        for i in range(NCORES)
    ]
    res = run_bass_kernel_spmd(nc, in_maps, core_ids=list(range(NCORES)))
    out = np.stack([res.results[i]["out"][:L] for i in range(NCORES)], axis=0)
    return out


if __name__ == "__main__":
    nc = build_kernel()
    print("build ok")

